# revision 62
# baseline (speedup 1.0000x reference)
"""MLA-style attention (nn_Attention_15496242004691) on 8 trn2 NeuronCores.

Strategy:
  Launch 1 (token-sharded, 512 tokens/core): cq = x@wq_a.T, ckv = x@wkv_a.T,
    RMSNorm of both (norm weights folded into the B projections on host),
    RoPE of k_pe (pair-swap folded into an extended wkv_a on host).
  Launch 2 (head-sharded, 2 heads/core): B projections (+q RoPE), causal
    attention with transposed scores (softmax column sums via GPSIMD
    partition_all_reduce over batched exp tiles), output projection; host
    sums the 8 partial outputs.

All tensors are bf16 except PSUM accumulation (fp32), the softmax
denominators (fp32) and the final output (fp32 partials summed on host).
Activations are kept feature-on-partition so no on-chip transposes occur.
DMAs are batched into few large transfers: descriptor generation (HWDGE) is
a serial ~625ns/DMA resource that a naive per-tile DMA schedule saturates.
"""

import numpy as np
import ml_dtypes

import concourse.bass as bass
import concourse.bass_isa as bass_isa
import concourse.mybir as mybir
import concourse.tile as tile
from concourse import bacc, library_config
from concourse.bass_utils import run_bass_kernel_spmd

F32 = mybir.dt.float32
F32R = mybir.dt.float32r
BF16 = mybir.dt.bfloat16
NPBF = ml_dtypes.bfloat16
AF = mybir.ActivationFunctionType
OP = mybir.AluOpType
RADD = bass_isa.ReduceOp.add

B, S, DIM, H = 2, 2048, 2048, 16
NCORES = 8
HPC = H // NCORES  # heads per core = 2
RQ = RKV = 512
DN, DR, DV, DQK = 128, 64, 128, 192
EPS = 1e-6
SCALE = DQK ** -0.5
T = B * S          # 4096 tokens
TS = T // NCORES   # 512 tokens per core in launch 1
ES_GROUP = 6       # exp tiles summed on DVE per partition_all_reduce

_CACHE = {}


# --------------------------------------------------------------------------
# Launch 1: A-projections + RMSNorm + k_pe RoPE (token-sharded)
# --------------------------------------------------------------------------
def build_k1():
    nc = bacc.Bacc("TRN2", target_bir_lowering=False)
    xt = nc.dram_tensor("xt", [DIM, TS], BF16, kind="ExternalInput")
    wqa = nc.dram_tensor("wqa", [128, 16, RQ], BF16, kind="ExternalInput")
    wkvam = nc.dram_tensor("wkvam", [128, 16, RKV], BF16, kind="ExternalInput")
    wpe = nc.dram_tensor("wpe", [128, 16 * 128], BF16, kind="ExternalInput")
    csk = nc.dram_tensor("csk", [128, TS], BF16, kind="ExternalInput")
    cqn = nc.dram_tensor("cqn", [RQ, TS], BF16, kind="ExternalOutput")
    ckvn = nc.dram_tensor("ckvn", [RKV, TS], BF16, kind="ExternalOutput")
    kpe = nc.dram_tensor("kpe", [DR, TS], BF16, kind="ExternalOutput")

    cqn_r = cqn[:, :].rearrange("(m p) t -> p m t", p=128)
    ckvn_r = ckvn[:, :].rearrange("(m p) t -> p m t", p=128)

    with tile.TileContext(nc) as tc:
        with tc.tile_pool(name="const", bufs=1) as cpool, \
             tc.tile_pool(name="sb", bufs=2) as sb, \
             tc.tile_pool(name="ps", bufs=1, space="PSUM") as ps:
            nc.gpsimd.load_library(library_config.attn)
            eps_t = cpool.tile([128, 1], F32)
            nc.vector.memset(eps_t, EPS)

            xt_t = cpool.tile([128, 16, TS], BF16)
            xt_r = xt[:, :].rearrange("(k p) t -> p k t", p=128)
            wqa_t = cpool.tile([128, 16, RQ], BF16)
            wkvam_t = cpool.tile([128, 16, RKV], BF16)
            wpe_t = cpool.tile([128, 16 * 128], BF16)
            cs_t = cpool.tile([128, TS], BF16)
            # DMA transfers serialize on the DMA engines, so stream the loads
            # in the exact order the k-major matmul loop consumes them
            for k2s, k2e in ((0, 2), (2, 4), (4, 8), (8, 16)):
                nc.sync.dma_start(out=wqa_t[:, k2s:k2e, :],
                                  in_=wqa[:, k2s:k2e, :])
                nc.sync.dma_start(out=xt_t[:, k2s:k2e, :],
                                  in_=xt_r[:, k2s:k2e, :])
            nc.sync.dma_start(out=wpe_t, in_=wpe[:, :])
            nc.sync.dma_start(out=wkvam_t[:, 0:8, :], in_=wkvam[:, 0:8, :])
            nc.sync.dma_start(out=cs_t, in_=csk[:, :])
            nc.sync.dma_start(out=wkvam_t[:, 8:16, :], in_=wkvam[:, 8:16, :])

            # q path first; kv gets fresh PSUM banks so its matmuls start
            # without waiting on q's evacuations (pe reuses q-m0's bank, the
            # first one freed)
            for path in ("q", "kv"):
                w_t = wqa_t if path == "q" else wkvam_t
                out_r = cqn_r if path == "q" else ckvn_r
                mtag = 0 if path == "q" else 4
                cqu = sb.tile([128, 4, TS], BF16, tag=f"cqu{path}", bufs=1)
                accs = []
                for m in range(4):
                    acc = ps.tile([128, TS], F32, tag=f"mm{mtag + m}", bufs=1)
                    accs.append(acc)
                if path == "kv":
                    # pe first in its own short k-loop: the RoPE chain then
                    # overlaps the kv latent matmuls instead of trailing
                    peacc = ps.tile([128, TS], F32, tag="mm0", bufs=1)
                    for k in range(16):
                        nc.tensor.matmul(peacc, wpe_t[:, k * 128:(k + 1) * 128],
                                         xt_t[:, k, :], start=(k == 0), stop=(k == 15))
                    pe_s = sb.tile([128, TS], BF16, tag="pes", bufs=1)
                    nc.scalar.copy(pe_s, peacc)
                    t0 = sb.tile([DR, TS], BF16, tag="t0", bufs=1)
                    t1 = sb.tile([DR, TS], BF16, tag="t1", bufs=1)
                    nc.vector.tensor_mul(t0, pe_s[0:DR, :], cs_t[0:DR, :])
                    nc.vector.tensor_mul(t1, pe_s[DR:128, :], cs_t[DR:128, :])
                    kp = sb.tile([DR, TS], BF16, tag="kp", bufs=1)
                    nc.vector.tensor_add(kp, t0, t1)
                    nc.sync.dma_start(out=kpe[:, :], in_=kp)
                # q phase streams behind the serialized DMA: k-major over all
                # four m so each arriving xt k-slice feeds 4 matmuls. kv phase
                # has all data resident, so it runs in m-pairs (first pair's
                # evacuation overlaps the second pair's matmuls) and its
                # normalize runs in column halves so the trailing chain's
                # per-op latency halves and the two chains pipeline.
                ncol = 2 if path == "kv" else 1
                CW = TS // ncol
                cols = [slice(c * CW, (c + 1) * CW) for c in range(ncol)]
                sqs = {}
                var_h = [[] for _ in range(ncol)]
                o_all = sb.tile([128, 4, TS], BF16, tag=f"oall{path}", bufs=1)
                halves = [(0, 1), (2, 3)] if path == "kv" else [(0, 1, 2, 3)]
                for half, ms in enumerate(halves):
                    for k in range(16):
                        for m in ms:
                            nc.tensor.matmul(accs[m],
                                             w_t[:, k, m * 128:(m + 1) * 128],
                                             xt_t[:, k, :],
                                             start=(k == 0), stop=(k == 15))
                    for c, cs_ in enumerate(cols):
                        for m in ms:
                            sq = sb.tile([128, CW], BF16, tag="sq", bufs=4)
                            if m % 2 == 1:
                                # evac via Act, square the SBUF copy on DVE
                                nc.scalar.copy(cqu[:, m, cs_], accs[m][:, cs_])
                                nc.vector.tensor_mul(sq, cqu[:, m, cs_],
                                                     cqu[:, m, cs_])
                            else:
                                nc.scalar.activation(sq, accs[m][:, cs_],
                                                     AF.Square)
                                nc.vector.tensor_copy(cqu[:, m, cs_],
                                                      accs[m][:, cs_])
                            sqs[(m, c)] = sq
                            if m % 2 == 1:
                                ss = sb.tile([128, CW], BF16, tag="ss", bufs=2)
                                nc.vector.tensor_add(ss, sqs[(m - 1, c)], sq)
                                vh = sb.tile([128, CW], F32, tag="vh", bufs=4)
                                nc.gpsimd.partition_all_reduce(
                                    vh, ss, channels=128, reduce_op=RADD)
                                var_h[c].append(vh)
                bcss = []
                for c, cs_ in enumerate(cols):
                    var_bc = sb.tile([128, CW], F32, tag="var", bufs=2)
                    nc.vector.tensor_add(var_bc, var_h[c][0], var_h[c][1])
                    bcs = sb.tile([128, CW], BF16, tag="bcs", bufs=2)
                    nc.scalar.activation(bcs, var_bc, AF.Abs_reciprocal_sqrt,
                                         scale=1.0 / 512.0, bias=eps_t[:, :])
                    bcss.append(bcs)
                # m-major muls with one merged DMA per m-pair: HWDGE is a
                # serial ~625ns/DMA device, so few big stores beat many small
                for m in range(4):
                    for c, cs_ in enumerate(cols):
                        nc.vector.tensor_mul(o_all[:, m, cs_], cqu[:, m, cs_],
                                             bcss[c])
                    if m % 2 == 1:
                        nc.sync.dma_start(out=out_r[:, m - 1:m + 1, :],
                                          in_=o_all[:, m - 1:m + 1, :])
    nc.compile()
    return nc


# --------------------------------------------------------------------------
# Launch 2: B-projections + q RoPE + causal attention + wo (head-sharded)
# --------------------------------------------------------------------------
def build_k2():
    nc = bacc.Bacc("TRN2", target_bir_lowering=False)
    cqn = nc.dram_tensor("cqn", [RQ, T], BF16, kind="ExternalInput")
    ckvn = nc.dram_tensor("ckvn", [RKV, T], BF16, kind="ExternalInput")
    kpe = nc.dram_tensor("kpe", [DR, T], BF16, kind="ExternalInput")
    wqb = nc.dram_tensor("wqb", [128, 4, 512], BF16, kind="ExternalInput")
    wkb = nc.dram_tensor("wkb", [128, 4, 256], BF16, kind="ExternalInput")
    wvb = nc.dram_tensor("wvb", [128, 4, 256], BF16, kind="ExternalInput")
    wop = nc.dram_tensor("wop", [128, 2, DIM], BF16, kind="ExternalInput")
    csf = nc.dram_tensor("csf", [128, S], BF16, kind="ExternalInput")
    trim = nc.dram_tensor("trim", [128, 128], BF16, kind="ExternalInput")
    out = nc.dram_tensor("out", [T, DIM], BF16, kind="ExternalOutput")

    cqn_r = cqn[:, :].rearrange("(k p) t -> p k t", p=128)
    ckvn_r = ckvn[:, :].rearrange("(k p) t -> p k t", p=128)

    with tile.TileContext(nc) as tc:
        with tc.tile_pool(name="const", bufs=1) as cpool, \
             tc.tile_pool(name="perb", bufs=1) as perb, \
             tc.tile_pool(name="sb", bufs=2) as sb, \
             tc.tile_pool(name="ps", bufs=1, space="PSUM") as ps:
            nc.gpsimd.load_library(library_config.attn)
            wqb_t = cpool.tile([128, 4, 512], BF16)
            wkb_t = cpool.tile([128, 4, 256], BF16)
            wvb_t = cpool.tile([128, 4, 256], BF16)
            wop_t = cpool.tile([128, 2, DIM], BF16)
            cs_t = cpool.tile([128, S], BF16)
            trim_t = cpool.tile([128, 128], BF16)

            consts_loaded = False

            # work deferred across heads/q-tiles/batches: trailing exp tiles,
            # output-projection chunks and softmax normalizes all drain while
            # the next block's score matmuls keep the PE busy (any PE stall
            # also resets its ~3us p-state ramp, costing ~1.5us extra)
            wo_queue = []
            wo_out_box = [None]
            pend_box = [None]
            pending = []

            def normalize(pend):
                hh, lst_p, oacc_p, qsl_p, o_ref = pend
                den_p = lst_p["den"]
                rec = sb.tile([128, 512], F32, tag="rec", bufs=2)
                nc.vector.reciprocal_approx_fast(rec, den_p)
                # two halves so dependent wo chunks can start on the first one
                q0 = qsl_p.start
                nc.vector.tensor_mul(o_ref[:, hh, q0:q0 + 256],
                                     oacc_p[:, 0:256], rec[:, 0:256])
                nc.vector.tensor_mul(o_ref[:, hh, q0 + 256:q0 + 512],
                                     oacc_p[:, 256:512], rec[:, 256:512])

            def wo_chunk(ent):
                bb, o_ref, t16, ch = ent
                tsl = slice(t16 * 128, (t16 + 1) * 128)
                acc = ps.tile([128, 512], F32, tag="mm", bufs=6)
                for hh in range(2):
                    nc.tensor.matmul(acc, o_ref[:, hh, tsl],
                                     wop_t[:, hh, ch * 512:(ch + 1) * 512],
                                     start=(hh == 0), stop=(hh == 1))
                if ch == 0:
                    outs = sb.tile([128, 2048], BF16, tag="outs", bufs=3)
                    wo_out_box[0] = outs
                else:
                    outs = wo_out_box[0]
                if ch % 2 == 0:
                    nc.scalar.copy(outs[:, ch * 512:(ch + 1) * 512], acc)
                else:
                    nc.vector.tensor_copy(outs[:, ch * 512:(ch + 1) * 512], acc)
                if ch % 2 == 1:
                    half = slice((ch - 1) * 512, (ch + 1) * 512)
                    nc.sync.dma_start(
                        out=out[bb * S + t16 * 128:bb * S + (t16 + 1) * 128, half],
                        in_=outs[:, half])

            for b in range(B):
                qn_t = perb.tile([128, 2, S], BF16, tag="qn")
                qp_t = perb.tile([DR, 2, S], BF16, tag="qp")
                kn_t = perb.tile([128, 2, S], BF16, tag="kn")
                kp_t = perb.tile([DR, S], BF16, tag="kp")
                v_t = perb.tile([128, 16, 256], BF16, tag="v", bufs=2)
                o_t = perb.tile([128, 2, S], BF16, tag="o", bufs=2)

                # ---- B projection of one 512-token tile ----
                def proj_tt(tt):
                    nonlocal consts_loaded
                    g0 = b * S + tt * 512
                    sl = slice(tt * 512, (tt + 1) * 512)
                    cq_t = sb.tile([128, 4, 512], BF16, tag="cq", bufs=2)
                    ckv_t = sb.tile([128, 4, 512], BF16, tag="ckv", bufs=2)
                    if not consts_loaded:
                        # stream the very first loads in k-slices so the
                        # k-major matmuls below start after ~2 slices, and
                        # order the rest by first use (kb needs wkb+ckv next,
                        # then wvb; cs only matters at the first evacuation)
                        for kk0, kk1 in ((0, 1), (1, 2), (2, 4)):
                            nc.sync.dma_start(out=wqb_t[:, kk0:kk1, :],
                                              in_=wqb[:, kk0:kk1, :])
                            nc.sync.dma_start(out=cq_t[:, kk0:kk1, :],
                                              in_=cqn_r[:, kk0:kk1, g0:g0 + 512])
                        nc.sync.dma_start(out=wkb_t, in_=wkb[:, :, :])
                        nc.sync.dma_start(out=cs_t[:, 0:512], in_=csf[:, 0:512])
                        nc.sync.dma_start(out=ckv_t, in_=ckvn_r[:, :, g0:g0 + 512])
                        nc.sync.dma_start(out=wvb_t, in_=wvb[:, :, :])
                        nc.sync.dma_start(out=cs_t[:, 512:2048], in_=csf[:, 512:2048])
                        consts_loaded = True
                    else:
                        nc.sync.dma_start(out=cq_t, in_=cqn_r[:, :, g0:g0 + 512])
                        nc.sync.dma_start(out=ckv_t, in_=ckvn_r[:, :, g0:g0 + 512])
                        if tt == 1 and b == 0:
                            nc.sync.dma_start(out=trim_t, in_=trim[:, :])
                        elif tt == 2 and b == 0:
                            nc.sync.dma_start(out=wop_t, in_=wop[:, :, :])
                    if tt == 0:
                        nc.sync.dma_start(out=kp_t, in_=kpe[:, b * S:(b + 1) * S])

                    # k-major so the first projection streams behind its loads
                    qaccs = []
                    for m in range(4):
                        qacc = ps.tile([128, 512], F32, tag="mm", bufs=6)
                        qaccs.append(qacc)
                    for k in range(4):
                        for m in range(4):
                            nc.tensor.matmul(qaccs[m],
                                             wqb_t[:, k, m * 128:(m + 1) * 128],
                                             cq_t[:, k, :], start=(k == 0), stop=(k == 3))
                    for m in range(4):  # h0 nope, h0 pe|swap, h1 nope, h1 pe|swap
                        acc = qaccs[m]
                        h = m // 2
                        if m % 2 == 0:
                            nc.vector.tensor_copy(qn_t[:, h, sl], acc)
                        else:
                            pe_s = sb.tile([128, 512], BF16, tag="pes", bufs=2)
                            nc.scalar.copy(pe_s, acc)
                            t0 = sb.tile([DR, 512], BF16, tag="t0", bufs=2)
                            t1 = sb.tile([DR, 512], BF16, tag="t1", bufs=2)
                            nc.vector.tensor_mul(t0, pe_s[0:DR, :], cs_t[0:DR, sl])
                            nc.vector.tensor_mul(t1, pe_s[DR:128, :], cs_t[DR:128, sl])
                            nc.vector.tensor_add(qp_t[:, h, sl], t0, t1)
                    for m in range(2):  # k_nope per head
                        acc = ps.tile([128, 512], F32, tag="mm", bufs=6)
                        for k in range(4):
                            nc.tensor.matmul(acc, wkb_t[:, k, m * 128:(m + 1) * 128],
                                             ckv_t[:, k, :], start=(k == 0), stop=(k == 3))
                        nc.scalar.copy(kn_t[:, m, sl], acc)
                    for tp in range(2):  # v, token-major, two 128-token halves
                        acc = ps.tile([128, 2, 256], F32, tag="mm", bufs=6)
                        for t4 in range(2):
                            for k in range(4):
                                nc.tensor.matmul(
                                    acc[:, t4, :],
                                    ckv_t[:, k, (2 * tp + t4) * 128:(2 * tp + t4 + 1) * 128],
                                    wvb_t[:, k, :], start=(k == 0), stop=(k == 3))
                        nc.vector.tensor_copy(v_t[:, tt * 4 + 2 * tp:tt * 4 + 2 * tp + 2, :],
                                              acc)

                # ---- causal attention (scores transposed: [k, q]) ----
                def attn_qt(qt):
                    for h in range(2):
                        qsl = slice(qt * 512, (qt + 1) * 512)
                        nkt = 4 * qt + 4
                        oacc = ps.tile([128, 512], F32, tag="pv", bufs=2)

                        # softmax denominators: exp tiles are zero-padded below
                        # their causal offset; DVE adds batch groups of
                        # ES_GROUP tiles and GPSIMD partition_all_reduce turns
                        # each group into broadcast column sums (fp32).
                        lst = {"pend": None, "es": None, "cnt": 0,
                               "den": None, "den_owned": False}

                        def close_group(last, lst=lst):
                            src = lst["es"] if lst["es"] is not None else lst["pend"]
                            deng = sb.tile([128, 512], F32, tag="deng", bufs=3)
                            nc.gpsimd.partition_all_reduce(deng, src, channels=128,
                                                           reduce_op=RADD)
                            if lst["den"] is None:
                                lst["den"] = deng
                            elif not lst["den_owned"]:
                                dacc = sb.tile([128, 512], F32, tag="den", bufs=3)
                                nc.vector.tensor_add(dacc, lst["den"], deng)
                                lst["den"] = dacc
                                lst["den_owned"] = True
                            else:
                                nc.vector.tensor_add(lst["den"], lst["den"], deng)
                            lst["pend"] = None
                            lst["es"] = None
                            lst["cnt"] = 0

                        def consume(prev_e, lst=lst, oacc=oacc, h=h, nkt=nkt,
                                    v_t=v_t, close_group=close_group):
                            et_p, off_p, kt_p = prev_e
                            last = (kt_p == nkt - 1)
                            nc.tensor.matmul(oacc[:, off_p:512],
                                             v_t[:, kt_p, h * 128:(h + 1) * 128],
                                             et_p[:, off_p:512],
                                             start=(kt_p == 0), stop=last)
                            if lst["pend"] is None and lst["es"] is None:
                                lst["pend"] = et_p
                                lst["cnt"] = 1
                            elif lst["es"] is None:
                                es = sb.tile([128, 512], BF16, tag="es", bufs=3)
                                nc.vector.tensor_add(es, lst["pend"], et_p)
                                lst["pend"] = None
                                lst["es"] = es
                                lst["cnt"] = 2
                            else:
                                nc.vector.tensor_add(lst["es"], lst["es"], et_p)
                                lst["cnt"] += 1
                            if last or lst["cnt"] >= ES_GROUP:
                                close_group(last)

                        for kt in range(nkt):
                            ksl = slice(kt * 128, (kt + 1) * 128)
                            j = kt - 4 * qt
                            # columns of this q-tile that can be unmasked:
                            off = 128 * j if j > 0 else 0
                            w = 512 - off
                            qs2 = slice(qt * 512 + off, (qt + 1) * 512)
                            sc = ps.tile([128, 512], F32, tag="mm", bufs=6)
                            nc.tensor.matmul(sc[:, :w], kn_t[:, h, ksl],
                                             qn_t[:, h, qs2], start=True, stop=False)
                            nc.tensor.matmul(sc[:, :w], kp_t[:, ksl],
                                             qp_t[:, h, qs2], start=False, stop=True)
                            if len(pending) >= 3:
                                fn, ent = pending.pop(0)
                                fn(ent)
                            et = sb.tile([128, 512], BF16, tag="exp", bufs=8)
                            if j > 0:
                                nc.gpsimd.memset(et[:, :off], 0.0)
                            nc.scalar.activation(et[:, off:512], sc[:, :w], AF.Exp,
                                                 scale=SCALE)
                            if 0 <= j < 4:
                                nc.vector.tensor_mul(et[:, off:off + 128],
                                                     et[:, off:off + 128], trim_t)
                            pending.append((consume, (et, off, kt)))
                            if kt == 3:
                                if pend_box[0] is not None:
                                    normalize(pend_box[0])
                                    pend_box[0] = None
                            elif kt >= 4 and wo_queue:
                                wo_chunk(wo_queue.pop(0))
                        pend_box[0] = (h, lst, oacc, qsl, o_t)
                        if h == 1:
                            wo_queue.extend([(b, o_t, t16, ch) for t16 in
                                             range(qt * 4, qt * 4 + 4) for ch in range(4)])

                # software pipeline: proj tiles feed attention one tile ahead
                proj_tt(0)
                proj_tt(1)
                attn_qt(0)
                proj_tt(2)
                attn_qt(1)
                proj_tt(3)
                attn_qt(2)
                attn_qt(3)

            for fn, ent in pending:
                fn(ent)
            pending.clear()
            if pend_box[0] is not None:
                normalize(pend_box[0])
                pend_box[0] = None
            for ent in wo_queue:
                wo_chunk(ent)

    nc.compile()
    return nc


# --------------------------------------------------------------------------
# Host-side data prep
# --------------------------------------------------------------------------
def _pack(wT, ktiles):
    """(ktiles*128, M) -> (128, ktiles, M) with [p, k, m] = wT[k*128+p, m]."""
    K, M = wT.shape
    assert K == ktiles * 128
    return np.ascontiguousarray(
        wT.reshape(ktiles, 128, M).transpose(1, 0, 2)).astype(NPBF)


def _swap_pairs(a, axis):
    idx = np.arange(a.shape[axis])
    idx = idx.reshape(-1, 2)[:, ::-1].reshape(-1)
    return np.take(a, idx, axis=axis)


def _prep(inputs):
    x = np.asarray(inputs["x"], dtype=np.float32)
    f = np.asarray(inputs["freqs_cis"], dtype=np.float32)
    wq_a = np.asarray(inputs["wq_a"], dtype=np.float32)
    wq_b = np.asarray(inputs["wq_b"], dtype=np.float32)
    q_norm_w = np.asarray(inputs["q_norm_w"], dtype=np.float32)
    wkv_a = np.asarray(inputs["wkv_a"], dtype=np.float32)
    kv_norm_w = np.asarray(inputs["kv_norm_w"], dtype=np.float32)
    wkv_b = np.asarray(inputs["wkv_b"], dtype=np.float32)
    wo = np.asarray(inputs["wo"], dtype=np.float32)

    xT = np.ascontiguousarray(x.reshape(T, DIM).T).astype(NPBF)  # (DIM, T)

    cos = f[:, :, 0].T  # (32, S)
    sin = f[:, :, 1].T
    cosF = np.empty((DR, S), np.float32)
    sinF = np.empty((DR, S), np.float32)
    cosF[0::2] = cos
    cosF[1::2] = cos
    sinF[0::2] = -sin
    sinF[1::2] = sin

    wqaT = wq_a.T                       # (DIM, RQ)
    wkvaT = wkv_a.T                     # (DIM, RKV+DR)
    pe = wkvaT[:, RKV:RKV + DR]
    pe_ext = np.concatenate([pe, _swap_pairs(pe, 1)], axis=1)   # (DIM, 128)
    wqa_p = _pack(wqaT, 16)
    wkvam_p = _pack(wkvaT[:, :RKV], 16)
    # wpe[p, k*128+j] = pe_ext[k*128+p, j] — contiguous 128-col blocks per k
    wpe_p = np.ascontiguousarray(
        pe_ext.reshape(16, 128, 128).transpose(1, 0, 2).reshape(128, 16 * 128)
    ).astype(NPBF)

    csB = np.concatenate([cosF, sinF], axis=0).astype(NPBF)  # (128, S)

    k1_maps = []
    for c in range(NCORES):
        t0 = c * TS
        srange = slice(t0 % S, t0 % S + TS)
        k1_maps.append({
            "xt": np.ascontiguousarray(xT[:, t0:t0 + TS]),
            "wqa": wqa_p, "wkvam": wkvam_p, "wpe": wpe_p,
            "csk": np.ascontiguousarray(csB[:, srange]),
        })

    # launch-2 per-core weights
    wqbT = (wq_b * q_norm_w[None, :]).T       # (RQ, H*DQK)
    wkvbT = (wkv_b * kv_norm_w[None, :]).T    # (RKV, H*(DN+DV))
    woT = wo.T                                # (H*DV, DIM)

    # strict-lower-triangle keep-mask for the 128x128 diagonal sub-block
    kk = np.arange(128)[:, None]
    qq = np.arange(128)[None, :]
    trim = (qq >= kk).astype(NPBF)

    csf = np.concatenate([cosF, sinF], axis=0).astype(NPBF)

    k2_maps = []
    for c in range(NCORES):
        h0, h1 = 2 * c, 2 * c + 1
        qcols = []
        for hh in (h0, h1):
            base = hh * DQK
            nope = wqbT[:, base:base + DN]
            pe_q = wqbT[:, base + DN:base + DQK]
            qcols += [nope, pe_q, _swap_pairs(pe_q, 1)]
        q_ext = np.concatenate(qcols, axis=1)             # (512, 512)
        kcols = [wkvbT[:, hh * (DN + DV):hh * (DN + DV) + DN] for hh in (h0, h1)]
        vcols = [wkvbT[:, hh * (DN + DV) + DN:(hh + 1) * (DN + DV)] for hh in (h0, h1)]
        worows = np.concatenate([woT[hh * DV:(hh + 1) * DV] for hh in (h0, h1)], axis=0)
        k2_maps.append({
            "wqb": _pack(q_ext, 4),
            "wkb": _pack(np.concatenate(kcols, axis=1), 4),
            "wvb": _pack(np.concatenate(vcols, axis=1), 4),
            "wop": _pack(worows, 2),
            "csf": csf, "trim": trim,
        })
    return k1_maps, k2_maps


def _get(name, builder):
    if name not in _CACHE:
        _CACHE[name] = builder()
    return _CACHE[name]


def _run(inputs, trace=False):
    k1_maps, k2_maps = _prep(inputs)
    nc1 = _get("k1", build_k1)
    r1 = run_bass_kernel_spmd(nc1, k1_maps, core_ids=list(range(NCORES)), trace=trace)

    cqn = np.concatenate([r1.results[c]["cqn"] for c in range(NCORES)], axis=1)
    ckvn = np.concatenate([r1.results[c]["ckvn"] for c in range(NCORES)], axis=1)
    kpe = np.concatenate([r1.results[c]["kpe"] for c in range(NCORES)], axis=1)
    for m in k2_maps:
        m["cqn"] = cqn
        m["ckvn"] = ckvn
        m["kpe"] = kpe

    nc2 = _get("k2", build_k2)
    r2 = run_bass_kernel_spmd(nc2, k2_maps, core_ids=list(range(NCORES)), trace=trace)

    acc = r2.results[0]["out"].astype(np.float32)
    for c in range(1, NCORES):
        acc = acc + r2.results[c]["out"]
    return acc.reshape(B, S, DIM), (r1, r2)


def kernel(**inputs) -> np.ndarray:
    out, _ = _run(inputs)
    return out


# revision 64
# speedup vs baseline: 1.0006x; 1.0006x over previous
"""MLA-style attention (nn_Attention_15496242004691) on 8 trn2 NeuronCores.

Strategy:
  Launch 1 (token-sharded, 512 tokens/core): cq = x@wq_a.T, ckv = x@wkv_a.T,
    RMSNorm of both (norm weights folded into the B projections on host),
    RoPE of k_pe (pair-swap folded into an extended wkv_a on host).
  Launch 2 (head-sharded, 2 heads/core): B projections (+q RoPE), causal
    attention with transposed scores (softmax column sums via GPSIMD
    partition_all_reduce over batched exp tiles), output projection; host
    sums the 8 partial outputs.

All tensors are bf16 except PSUM accumulation (fp32), the softmax
denominators (fp32) and the final output (fp32 partials summed on host).
Activations are kept feature-on-partition so no on-chip transposes occur.
DMAs are batched into few large transfers: descriptor generation (HWDGE) is
a serial ~625ns/DMA resource that a naive per-tile DMA schedule saturates.
"""

import numpy as np
import ml_dtypes

import concourse.bass as bass
import concourse.bass_isa as bass_isa
import concourse.mybir as mybir
import concourse.tile as tile
from concourse import bacc, library_config
from concourse.bass_utils import run_bass_kernel_spmd

F32 = mybir.dt.float32
F32R = mybir.dt.float32r
BF16 = mybir.dt.bfloat16
NPBF = ml_dtypes.bfloat16
AF = mybir.ActivationFunctionType
OP = mybir.AluOpType
RADD = bass_isa.ReduceOp.add

B, S, DIM, H = 2, 2048, 2048, 16
NCORES = 8
HPC = H // NCORES  # heads per core = 2
RQ = RKV = 512
DN, DR, DV, DQK = 128, 64, 128, 192
EPS = 1e-6
SCALE = DQK ** -0.5
T = B * S          # 4096 tokens
TS = T // NCORES   # 512 tokens per core in launch 1
ES_GROUP = 6       # exp tiles summed on DVE per partition_all_reduce

_CACHE = {}


# --------------------------------------------------------------------------
# Launch 1: A-projections + RMSNorm + k_pe RoPE (token-sharded)
# --------------------------------------------------------------------------
def build_k1():
    nc = bacc.Bacc("TRN2", target_bir_lowering=False)
    xt = nc.dram_tensor("xt", [DIM, TS], BF16, kind="ExternalInput")
    wqa = nc.dram_tensor("wqa", [128, 16, RQ], BF16, kind="ExternalInput")
    wkvam = nc.dram_tensor("wkvam", [128, 16, RKV], BF16, kind="ExternalInput")
    wpe = nc.dram_tensor("wpe", [128, 16 * 128], BF16, kind="ExternalInput")
    csk = nc.dram_tensor("csk", [128, TS], BF16, kind="ExternalInput")
    cqn = nc.dram_tensor("cqn", [RQ, TS], BF16, kind="ExternalOutput")
    ckvn = nc.dram_tensor("ckvn", [RKV, TS], BF16, kind="ExternalOutput")
    kpe = nc.dram_tensor("kpe", [DR, TS], BF16, kind="ExternalOutput")

    cqn_r = cqn[:, :].rearrange("(m p) t -> p m t", p=128)
    ckvn_r = ckvn[:, :].rearrange("(m p) t -> p m t", p=128)

    with tile.TileContext(nc) as tc:
        with tc.tile_pool(name="const", bufs=1) as cpool, \
             tc.tile_pool(name="sb", bufs=2) as sb, \
             tc.tile_pool(name="ps", bufs=1, space="PSUM") as ps:
            nc.gpsimd.load_library(library_config.attn)
            eps_t = cpool.tile([128, 1], F32)
            nc.vector.memset(eps_t, EPS)

            xt_t = cpool.tile([128, 16, TS], BF16)
            xt_r = xt[:, :].rearrange("(k p) t -> p k t", p=128)
            wqa_t = cpool.tile([128, 16, RQ], BF16)
            wkvam_t = cpool.tile([128, 16, RKV], BF16)
            wpe_t = cpool.tile([128, 16 * 128], BF16)
            cs_t = cpool.tile([128, TS], BF16)
            # DMA transfers serialize on the DMA engines, so stream the loads
            # in the exact order the k-major matmul loop consumes them
            for k2s, k2e in ((0, 2), (2, 4), (4, 8), (8, 16)):
                nc.sync.dma_start(out=wqa_t[:, k2s:k2e, :],
                                  in_=wqa[:, k2s:k2e, :])
                nc.sync.dma_start(out=xt_t[:, k2s:k2e, :],
                                  in_=xt_r[:, k2s:k2e, :])
            nc.sync.dma_start(out=wpe_t, in_=wpe[:, :])
            nc.sync.dma_start(out=wkvam_t[:, 0:8, :], in_=wkvam[:, 0:8, :])
            nc.sync.dma_start(out=cs_t, in_=csk[:, :])
            nc.sync.dma_start(out=wkvam_t[:, 8:16, :], in_=wkvam[:, 8:16, :])

            # q path first; kv gets fresh PSUM banks so its matmuls start
            # without waiting on q's evacuations (pe reuses q-m0's bank, the
            # first one freed)
            for path in ("q", "kv"):
                w_t = wqa_t if path == "q" else wkvam_t
                out_r = cqn_r if path == "q" else ckvn_r
                mtag = 0 if path == "q" else 4
                cqu = sb.tile([128, 4, TS], BF16, tag=f"cqu{path}", bufs=1)
                accs = []
                for m in range(4):
                    acc = ps.tile([128, TS], F32, tag=f"mm{mtag + m}", bufs=1)
                    accs.append(acc)
                if path == "kv":
                    # pe first in its own short k-loop: the RoPE chain then
                    # overlaps the kv latent matmuls instead of trailing
                    peacc = ps.tile([128, TS], F32, tag="mm0", bufs=1)
                    for k in range(16):
                        nc.tensor.matmul(peacc, wpe_t[:, k * 128:(k + 1) * 128],
                                         xt_t[:, k, :], start=(k == 0), stop=(k == 15))
                    pe_s = sb.tile([128, TS], BF16, tag="pes", bufs=1)
                    nc.scalar.copy(pe_s, peacc)
                    t0 = sb.tile([DR, TS], BF16, tag="t0", bufs=1)
                    t1 = sb.tile([DR, TS], BF16, tag="t1", bufs=1)
                    nc.vector.tensor_mul(t0, pe_s[0:DR, :], cs_t[0:DR, :])
                    nc.vector.tensor_mul(t1, pe_s[DR:128, :], cs_t[DR:128, :])
                    kp = sb.tile([DR, TS], BF16, tag="kp", bufs=1)
                    nc.vector.tensor_add(kp, t0, t1)
                    nc.sync.dma_start(out=kpe[:, :], in_=kp)
                # q phase streams behind the serialized DMA: k-major over all
                # four m so each arriving xt k-slice feeds 4 matmuls. kv phase
                # has all data resident, so it runs in m-pairs (first pair's
                # evacuation overlaps the second pair's matmuls) and its
                # normalize runs in column halves so the trailing chain's
                # per-op latency halves and the two chains pipeline.
                ncol = 2 if path == "kv" else 1
                CW = TS // ncol
                cols = [slice(c * CW, (c + 1) * CW) for c in range(ncol)]
                sqs = {}
                var_h = [[] for _ in range(ncol)]
                o_all = sb.tile([128, 4, TS], BF16, tag=f"oall{path}", bufs=1)
                halves = [(0, 1), (2,), (3,)] if path == "kv" else [(0, 1, 2, 3)]
                for half, ms in enumerate(halves):
                    for k in range(16):
                        for m in ms:
                            nc.tensor.matmul(accs[m],
                                             w_t[:, k, m * 128:(m + 1) * 128],
                                             xt_t[:, k, :],
                                             start=(k == 0), stop=(k == 15))
                    for c, cs_ in enumerate(cols):
                        for m in ms:
                            sq = sb.tile([128, CW], BF16, tag="sq", bufs=4)
                            if m % 2 == 1:
                                # evac via Act, square the SBUF copy on DVE
                                nc.scalar.copy(cqu[:, m, cs_], accs[m][:, cs_])
                                nc.vector.tensor_mul(sq, cqu[:, m, cs_],
                                                     cqu[:, m, cs_])
                            else:
                                nc.scalar.activation(sq, accs[m][:, cs_],
                                                     AF.Square)
                                nc.vector.tensor_copy(cqu[:, m, cs_],
                                                      accs[m][:, cs_])
                            sqs[(m, c)] = sq
                            if m % 2 == 1:
                                ss = sb.tile([128, CW], BF16, tag="ss", bufs=2)
                                nc.vector.tensor_add(ss, sqs[(m - 1, c)], sq)
                                vh = sb.tile([128, CW], F32, tag="vh", bufs=4)
                                nc.gpsimd.partition_all_reduce(
                                    vh, ss, channels=128, reduce_op=RADD)
                                var_h[c].append(vh)
                bcss = []
                for c, cs_ in enumerate(cols):
                    var_bc = sb.tile([128, CW], F32, tag="var", bufs=2)
                    nc.vector.tensor_add(var_bc, var_h[c][0], var_h[c][1])
                    bcs = sb.tile([128, CW], BF16, tag="bcs", bufs=2)
                    nc.scalar.activation(bcs, var_bc, AF.Abs_reciprocal_sqrt,
                                         scale=1.0 / 512.0, bias=eps_t[:, :])
                    bcss.append(bcs)
                # m-major muls with one merged DMA per m-pair (the last pair
                # stores per-m so the final transfer starts earlier): HWDGE is
                # a serial ~625ns/DMA device, so keep store count low
                for m in range(4):
                    for c, cs_ in enumerate(cols):
                        nc.vector.tensor_mul(o_all[:, m, cs_], cqu[:, m, cs_],
                                             bcss[c])
                    if m == 1:
                        nc.sync.dma_start(out=out_r[:, 0:2, :],
                                          in_=o_all[:, 0:2, :])
                    elif m >= 2:
                        nc.sync.dma_start(out=out_r[:, m, :],
                                          in_=o_all[:, m, :])
    nc.compile()
    return nc


# --------------------------------------------------------------------------
# Launch 2: B-projections + q RoPE + causal attention + wo (head-sharded)
# --------------------------------------------------------------------------
def build_k2():
    nc = bacc.Bacc("TRN2", target_bir_lowering=False)
    cqn = nc.dram_tensor("cqn", [RQ, T], BF16, kind="ExternalInput")
    ckvn = nc.dram_tensor("ckvn", [RKV, T], BF16, kind="ExternalInput")
    kpe = nc.dram_tensor("kpe", [DR, T], BF16, kind="ExternalInput")
    wqb = nc.dram_tensor("wqb", [128, 4, 512], BF16, kind="ExternalInput")
    wkb = nc.dram_tensor("wkb", [128, 4, 256], BF16, kind="ExternalInput")
    wvb = nc.dram_tensor("wvb", [128, 4, 256], BF16, kind="ExternalInput")
    wop = nc.dram_tensor("wop", [128, 2, DIM], BF16, kind="ExternalInput")
    csf = nc.dram_tensor("csf", [128, S], BF16, kind="ExternalInput")
    trim = nc.dram_tensor("trim", [128, 128], BF16, kind="ExternalInput")
    out = nc.dram_tensor("out", [T, DIM], BF16, kind="ExternalOutput")

    cqn_r = cqn[:, :].rearrange("(k p) t -> p k t", p=128)
    ckvn_r = ckvn[:, :].rearrange("(k p) t -> p k t", p=128)

    with tile.TileContext(nc) as tc:
        with tc.tile_pool(name="const", bufs=1) as cpool, \
             tc.tile_pool(name="perb", bufs=1) as perb, \
             tc.tile_pool(name="sb", bufs=2) as sb, \
             tc.tile_pool(name="ps", bufs=1, space="PSUM") as ps:
            nc.gpsimd.load_library(library_config.attn)
            wqb_t = cpool.tile([128, 4, 512], BF16)
            wkb_t = cpool.tile([128, 4, 256], BF16)
            wvb_t = cpool.tile([128, 4, 256], BF16)
            wop_t = cpool.tile([128, 2, DIM], BF16)
            cs_t = cpool.tile([128, S], BF16)
            trim_t = cpool.tile([128, 128], BF16)

            consts_loaded = False

            # work deferred across heads/q-tiles/batches: trailing exp tiles,
            # output-projection chunks and softmax normalizes all drain while
            # the next block's score matmuls keep the PE busy (any PE stall
            # also resets its ~3us p-state ramp, costing ~1.5us extra)
            wo_queue = []
            wo_out_box = [None]
            pend_box = [None]
            pending = []

            def normalize(pend):
                hh, lst_p, oacc_p, qsl_p, o_ref = pend
                den_p = lst_p["den"]
                rec = sb.tile([128, 512], F32, tag="rec", bufs=2)
                nc.vector.reciprocal_approx_fast(rec, den_p)
                # two halves so dependent wo chunks can start on the first one
                q0 = qsl_p.start
                nc.vector.tensor_mul(o_ref[:, hh, q0:q0 + 256],
                                     oacc_p[:, 0:256], rec[:, 0:256])
                nc.vector.tensor_mul(o_ref[:, hh, q0 + 256:q0 + 512],
                                     oacc_p[:, 256:512], rec[:, 256:512])

            def wo_chunk(ent):
                bb, o_ref, t16, ch = ent
                tsl = slice(t16 * 128, (t16 + 1) * 128)
                acc = ps.tile([128, 512], F32, tag="mm", bufs=6)
                for hh in range(2):
                    nc.tensor.matmul(acc, o_ref[:, hh, tsl],
                                     wop_t[:, hh, ch * 512:(ch + 1) * 512],
                                     start=(hh == 0), stop=(hh == 1))
                if ch == 0:
                    outs = sb.tile([128, 2048], BF16, tag="outs", bufs=3)
                    wo_out_box[0] = outs
                else:
                    outs = wo_out_box[0]
                if ch % 2 == 0:
                    nc.scalar.copy(outs[:, ch * 512:(ch + 1) * 512], acc)
                else:
                    nc.vector.tensor_copy(outs[:, ch * 512:(ch + 1) * 512], acc)
                if ch % 2 == 1:
                    half = slice((ch - 1) * 512, (ch + 1) * 512)
                    nc.sync.dma_start(
                        out=out[bb * S + t16 * 128:bb * S + (t16 + 1) * 128, half],
                        in_=outs[:, half])

            for b in range(B):
                qn_t = perb.tile([128, 2, S], BF16, tag="qn")
                qp_t = perb.tile([DR, 2, S], BF16, tag="qp")
                kn_t = perb.tile([128, 2, S], BF16, tag="kn")
                kp_t = perb.tile([DR, S], BF16, tag="kp")
                v_t = perb.tile([128, 16, 256], BF16, tag="v", bufs=2)
                o_t = perb.tile([128, 2, S], BF16, tag="o", bufs=2)

                # ---- B projection of one 512-token tile ----
                def proj_tt(tt):
                    nonlocal consts_loaded
                    g0 = b * S + tt * 512
                    sl = slice(tt * 512, (tt + 1) * 512)
                    cq_t = sb.tile([128, 4, 512], BF16, tag="cq", bufs=2)
                    ckv_t = sb.tile([128, 4, 512], BF16, tag="ckv", bufs=2)
                    if not consts_loaded:
                        # stream the very first loads in k-slices so the
                        # k-major matmuls below start after ~2 slices, and
                        # order the rest by first use (kb needs wkb+ckv next,
                        # then wvb; cs only matters at the first evacuation)
                        for kk0, kk1 in ((0, 1), (1, 2), (2, 4)):
                            nc.sync.dma_start(out=wqb_t[:, kk0:kk1, :],
                                              in_=wqb[:, kk0:kk1, :])
                            nc.sync.dma_start(out=cq_t[:, kk0:kk1, :],
                                              in_=cqn_r[:, kk0:kk1, g0:g0 + 512])
                        nc.sync.dma_start(out=wkb_t, in_=wkb[:, :, :])
                        nc.sync.dma_start(out=cs_t[:, 0:512], in_=csf[:, 0:512])
                        nc.sync.dma_start(out=ckv_t, in_=ckvn_r[:, :, g0:g0 + 512])
                        nc.sync.dma_start(out=wvb_t, in_=wvb[:, :, :])
                        nc.sync.dma_start(out=cs_t[:, 512:2048], in_=csf[:, 512:2048])
                        consts_loaded = True
                    else:
                        nc.sync.dma_start(out=cq_t, in_=cqn_r[:, :, g0:g0 + 512])
                        nc.sync.dma_start(out=ckv_t, in_=ckvn_r[:, :, g0:g0 + 512])
                        if tt == 1 and b == 0:
                            nc.sync.dma_start(out=trim_t, in_=trim[:, :])
                        elif tt == 2 and b == 0:
                            nc.sync.dma_start(out=wop_t, in_=wop[:, :, :])
                    if tt == 0:
                        nc.sync.dma_start(out=kp_t, in_=kpe[:, b * S:(b + 1) * S])

                    # k-major so the first projection streams behind its loads
                    qaccs = []
                    for m in range(4):
                        qacc = ps.tile([128, 512], F32, tag="mm", bufs=6)
                        qaccs.append(qacc)
                    for k in range(4):
                        for m in range(4):
                            nc.tensor.matmul(qaccs[m],
                                             wqb_t[:, k, m * 128:(m + 1) * 128],
                                             cq_t[:, k, :], start=(k == 0), stop=(k == 3))
                    for m in range(4):  # h0 nope, h0 pe|swap, h1 nope, h1 pe|swap
                        acc = qaccs[m]
                        h = m // 2
                        if m % 2 == 0:
                            nc.vector.tensor_copy(qn_t[:, h, sl], acc)
                        else:
                            pe_s = sb.tile([128, 512], BF16, tag="pes", bufs=2)
                            nc.scalar.copy(pe_s, acc)
                            t0 = sb.tile([DR, 512], BF16, tag="t0", bufs=2)
                            t1 = sb.tile([DR, 512], BF16, tag="t1", bufs=2)
                            nc.vector.tensor_mul(t0, pe_s[0:DR, :], cs_t[0:DR, sl])
                            nc.vector.tensor_mul(t1, pe_s[DR:128, :], cs_t[DR:128, sl])
                            nc.vector.tensor_add(qp_t[:, h, sl], t0, t1)
                    for m in range(2):  # k_nope per head
                        acc = ps.tile([128, 512], F32, tag="mm", bufs=6)
                        for k in range(4):
                            nc.tensor.matmul(acc, wkb_t[:, k, m * 128:(m + 1) * 128],
                                             ckv_t[:, k, :], start=(k == 0), stop=(k == 3))
                        nc.scalar.copy(kn_t[:, m, sl], acc)
                    for tp in range(2):  # v, token-major, two 128-token halves
                        acc = ps.tile([128, 2, 256], F32, tag="mm", bufs=6)
                        for t4 in range(2):
                            for k in range(4):
                                nc.tensor.matmul(
                                    acc[:, t4, :],
                                    ckv_t[:, k, (2 * tp + t4) * 128:(2 * tp + t4 + 1) * 128],
                                    wvb_t[:, k, :], start=(k == 0), stop=(k == 3))
                        nc.vector.tensor_copy(v_t[:, tt * 4 + 2 * tp:tt * 4 + 2 * tp + 2, :],
                                              acc)

                # ---- causal attention (scores transposed: [k, q]) ----
                def attn_qt(qt):
                    for h in range(2):
                        qsl = slice(qt * 512, (qt + 1) * 512)
                        nkt = 4 * qt + 4
                        oacc = ps.tile([128, 512], F32, tag="pv", bufs=2)

                        # softmax denominators: exp tiles are zero-padded below
                        # their causal offset; DVE adds batch groups of
                        # ES_GROUP tiles and GPSIMD partition_all_reduce turns
                        # each group into broadcast column sums (fp32).
                        lst = {"pend": None, "es": None, "cnt": 0,
                               "den": None, "den_owned": False}

                        def close_group(last, lst=lst):
                            src = lst["es"] if lst["es"] is not None else lst["pend"]
                            deng = sb.tile([128, 512], F32, tag="deng", bufs=3)
                            nc.gpsimd.partition_all_reduce(deng, src, channels=128,
                                                           reduce_op=RADD)
                            if lst["den"] is None:
                                lst["den"] = deng
                            elif not lst["den_owned"]:
                                dacc = sb.tile([128, 512], F32, tag="den", bufs=3)
                                nc.vector.tensor_add(dacc, lst["den"], deng)
                                lst["den"] = dacc
                                lst["den_owned"] = True
                            else:
                                nc.vector.tensor_add(lst["den"], lst["den"], deng)
                            lst["pend"] = None
                            lst["es"] = None
                            lst["cnt"] = 0

                        def consume(prev_e, lst=lst, oacc=oacc, h=h, nkt=nkt,
                                    v_t=v_t, close_group=close_group):
                            et_p, off_p, kt_p = prev_e
                            last = (kt_p == nkt - 1)
                            nc.tensor.matmul(oacc[:, off_p:512],
                                             v_t[:, kt_p, h * 128:(h + 1) * 128],
                                             et_p[:, off_p:512],
                                             start=(kt_p == 0), stop=last)
                            if lst["pend"] is None and lst["es"] is None:
                                lst["pend"] = et_p
                                lst["cnt"] = 1
                            elif lst["es"] is None:
                                es = sb.tile([128, 512], BF16, tag="es", bufs=3)
                                nc.vector.tensor_add(es, lst["pend"], et_p)
                                lst["pend"] = None
                                lst["es"] = es
                                lst["cnt"] = 2
                            else:
                                nc.vector.tensor_add(lst["es"], lst["es"], et_p)
                                lst["cnt"] += 1
                            if last or lst["cnt"] >= ES_GROUP:
                                close_group(last)

                        for kt in range(nkt):
                            ksl = slice(kt * 128, (kt + 1) * 128)
                            j = kt - 4 * qt
                            # columns of this q-tile that can be unmasked:
                            off = 128 * j if j > 0 else 0
                            w = 512 - off
                            qs2 = slice(qt * 512 + off, (qt + 1) * 512)
                            sc = ps.tile([128, 512], F32, tag="mm", bufs=6)
                            nc.tensor.matmul(sc[:, :w], kn_t[:, h, ksl],
                                             qn_t[:, h, qs2], start=True, stop=False)
                            nc.tensor.matmul(sc[:, :w], kp_t[:, ksl],
                                             qp_t[:, h, qs2], start=False, stop=True)
                            if len(pending) >= 3:
                                fn, ent = pending.pop(0)
                                fn(ent)
                            et = sb.tile([128, 512], BF16, tag="exp", bufs=8)
                            if j > 0:
                                nc.gpsimd.memset(et[:, :off], 0.0)
                            nc.scalar.activation(et[:, off:512], sc[:, :w], AF.Exp,
                                                 scale=SCALE)
                            if 0 <= j < 4:
                                nc.vector.tensor_mul(et[:, off:off + 128],
                                                     et[:, off:off + 128], trim_t)
                            pending.append((consume, (et, off, kt)))
                            if kt == 3:
                                if pend_box[0] is not None:
                                    normalize(pend_box[0])
                                    pend_box[0] = None
                            elif kt >= 4 and wo_queue:
                                wo_chunk(wo_queue.pop(0))
                        pend_box[0] = (h, lst, oacc, qsl, o_t)
                        if h == 1:
                            wo_queue.extend([(b, o_t, t16, ch) for t16 in
                                             range(qt * 4, qt * 4 + 4) for ch in range(4)])

                # software pipeline: proj tiles feed attention one tile ahead
                proj_tt(0)
                proj_tt(1)
                attn_qt(0)
                proj_tt(2)
                attn_qt(1)
                proj_tt(3)
                attn_qt(2)
                attn_qt(3)

            for fn, ent in pending:
                fn(ent)
            pending.clear()
            if pend_box[0] is not None:
                normalize(pend_box[0])
                pend_box[0] = None
            for ent in wo_queue:
                wo_chunk(ent)

    nc.compile()
    return nc


# --------------------------------------------------------------------------
# Host-side data prep
# --------------------------------------------------------------------------
def _pack(wT, ktiles):
    """(ktiles*128, M) -> (128, ktiles, M) with [p, k, m] = wT[k*128+p, m]."""
    K, M = wT.shape
    assert K == ktiles * 128
    return np.ascontiguousarray(
        wT.reshape(ktiles, 128, M).transpose(1, 0, 2)).astype(NPBF)


def _swap_pairs(a, axis):
    idx = np.arange(a.shape[axis])
    idx = idx.reshape(-1, 2)[:, ::-1].reshape(-1)
    return np.take(a, idx, axis=axis)


def _prep(inputs):
    x = np.asarray(inputs["x"], dtype=np.float32)
    f = np.asarray(inputs["freqs_cis"], dtype=np.float32)
    wq_a = np.asarray(inputs["wq_a"], dtype=np.float32)
    wq_b = np.asarray(inputs["wq_b"], dtype=np.float32)
    q_norm_w = np.asarray(inputs["q_norm_w"], dtype=np.float32)
    wkv_a = np.asarray(inputs["wkv_a"], dtype=np.float32)
    kv_norm_w = np.asarray(inputs["kv_norm_w"], dtype=np.float32)
    wkv_b = np.asarray(inputs["wkv_b"], dtype=np.float32)
    wo = np.asarray(inputs["wo"], dtype=np.float32)

    xT = np.ascontiguousarray(x.reshape(T, DIM).T).astype(NPBF)  # (DIM, T)

    cos = f[:, :, 0].T  # (32, S)
    sin = f[:, :, 1].T
    cosF = np.empty((DR, S), np.float32)
    sinF = np.empty((DR, S), np.float32)
    cosF[0::2] = cos
    cosF[1::2] = cos
    sinF[0::2] = -sin
    sinF[1::2] = sin

    wqaT = wq_a.T                       # (DIM, RQ)
    wkvaT = wkv_a.T                     # (DIM, RKV+DR)
    pe = wkvaT[:, RKV:RKV + DR]
    pe_ext = np.concatenate([pe, _swap_pairs(pe, 1)], axis=1)   # (DIM, 128)
    wqa_p = _pack(wqaT, 16)
    wkvam_p = _pack(wkvaT[:, :RKV], 16)
    # wpe[p, k*128+j] = pe_ext[k*128+p, j] — contiguous 128-col blocks per k
    wpe_p = np.ascontiguousarray(
        pe_ext.reshape(16, 128, 128).transpose(1, 0, 2).reshape(128, 16 * 128)
    ).astype(NPBF)

    csB = np.concatenate([cosF, sinF], axis=0).astype(NPBF)  # (128, S)

    k1_maps = []
    for c in range(NCORES):
        t0 = c * TS
        srange = slice(t0 % S, t0 % S + TS)
        k1_maps.append({
            "xt": np.ascontiguousarray(xT[:, t0:t0 + TS]),
            "wqa": wqa_p, "wkvam": wkvam_p, "wpe": wpe_p,
            "csk": np.ascontiguousarray(csB[:, srange]),
        })

    # launch-2 per-core weights
    wqbT = (wq_b * q_norm_w[None, :]).T       # (RQ, H*DQK)
    wkvbT = (wkv_b * kv_norm_w[None, :]).T    # (RKV, H*(DN+DV))
    woT = wo.T                                # (H*DV, DIM)

    # strict-lower-triangle keep-mask for the 128x128 diagonal sub-block
    kk = np.arange(128)[:, None]
    qq = np.arange(128)[None, :]
    trim = (qq >= kk).astype(NPBF)

    csf = np.concatenate([cosF, sinF], axis=0).astype(NPBF)

    k2_maps = []
    for c in range(NCORES):
        h0, h1 = 2 * c, 2 * c + 1
        qcols = []
        for hh in (h0, h1):
            base = hh * DQK
            nope = wqbT[:, base:base + DN]
            pe_q = wqbT[:, base + DN:base + DQK]
            qcols += [nope, pe_q, _swap_pairs(pe_q, 1)]
        q_ext = np.concatenate(qcols, axis=1)             # (512, 512)
        kcols = [wkvbT[:, hh * (DN + DV):hh * (DN + DV) + DN] for hh in (h0, h1)]
        vcols = [wkvbT[:, hh * (DN + DV) + DN:(hh + 1) * (DN + DV)] for hh in (h0, h1)]
        worows = np.concatenate([woT[hh * DV:(hh + 1) * DV] for hh in (h0, h1)], axis=0)
        k2_maps.append({
            "wqb": _pack(q_ext, 4),
            "wkb": _pack(np.concatenate(kcols, axis=1), 4),
            "wvb": _pack(np.concatenate(vcols, axis=1), 4),
            "wop": _pack(worows, 2),
            "csf": csf, "trim": trim,
        })
    return k1_maps, k2_maps


def _get(name, builder):
    if name not in _CACHE:
        _CACHE[name] = builder()
    return _CACHE[name]


def _run(inputs, trace=False):
    k1_maps, k2_maps = _prep(inputs)
    nc1 = _get("k1", build_k1)
    r1 = run_bass_kernel_spmd(nc1, k1_maps, core_ids=list(range(NCORES)), trace=trace)

    cqn = np.concatenate([r1.results[c]["cqn"] for c in range(NCORES)], axis=1)
    ckvn = np.concatenate([r1.results[c]["ckvn"] for c in range(NCORES)], axis=1)
    kpe = np.concatenate([r1.results[c]["kpe"] for c in range(NCORES)], axis=1)
    for m in k2_maps:
        m["cqn"] = cqn
        m["ckvn"] = ckvn
        m["kpe"] = kpe

    nc2 = _get("k2", build_k2)
    r2 = run_bass_kernel_spmd(nc2, k2_maps, core_ids=list(range(NCORES)), trace=trace)

    acc = r2.results[0]["out"].astype(np.float32)
    for c in range(1, NCORES):
        acc = acc + r2.results[c]["out"]
    return acc.reshape(B, S, DIM), (r1, r2)


def kernel(**inputs) -> np.ndarray:
    out, _ = _run(inputs)
    return out
